# revision 52
# baseline (speedup 1.0000x reference)
"""Causal self-attention (LN + QKV + causal MHA + proj) on 8 TRN2 NeuronCores.

Sharding: tensor-parallel over heads. 16 heads / 8 cores = 2 heads per core.
Each core computes LN stats + its QKV column slice + attention for its 2
heads + its row-slice of the output projection; partial proj outputs are
summed on the host.

Matmul operand dtype is switchable (bf16 default, fp32r fallback).
"""

import os
from contextlib import ExitStack

import ml_dtypes
import numpy as np

import concourse.bass as bass
import concourse.tile as tile
from concourse import bacc, mybir
from concourse.bass_utils import run_bass_kernel_spmd

# Problem shape (hardcoded per contract).
B, T = 4, 2048
N_EMBD = 1024
C_IN = 1152
N_HEAD = 16
HD = 64
N_CORES = 8
HPC = N_HEAD // N_CORES  # heads per core = 2
BT = B * T  # 8192
CC = C_IN // 128  # 9 contraction chunks
TCH_PER_B = T // 128  # 16
QT = 512  # q tile
NJT = T // QT  # 4 q tiles per b
COLS = 3 * HPC * HD  # 384 qkv cols per core
EPS = 1e-5

F32 = mybir.dt.float32
F32R = mybir.dt.float32r
BF16 = mybir.dt.bfloat16

# matmul operand dtype: "bf16" or "f32r" (env-overridable for experiments)
# "bf16": everything bf16; "f32r": everything fp32r;
# "hyb": QKV matmul inputs fp32r (x quantization dominates error), rest bf16
MM_MODE = os.environ.get("KMM_MODE", "bf16")
if MM_MODE == "f32r":
    MMDT, MMNP = F32R, np.float32  # attention/proj operands
    QDT, QNP = F32R, np.float32  # qkv matmul operands (xt, w_attn)
elif MM_MODE == "hyb":
    MMDT, MMNP = BF16, ml_dtypes.bfloat16
    QDT, QNP = F32R, np.float32
else:
    MMDT, MMNP = BF16, ml_dtypes.bfloat16
    QDT, QNP = BF16, ml_dtypes.bfloat16
K_EXACT = os.environ.get("K_EXACT", "1") == "1"  # exact-width exp + diag mask
# reciprocal_approx_fast (custom DVE op) produces wrong results in this
# toolchain — keep the plain InstReciprocal.
K_FASTRECIP = os.environ.get("K_FASTRECIP", "0") == "1"
K_DBG = os.environ.get("K_DBG", "0") == "1"  # dump b=0 intermediates

LAST_RESULTS = None  # test harness reads exec_time from here
_CACHED_NC = None


def _magic_rsqrt(nc, pool, vpe, n):
    """rstd = 1/sqrt(vpe) for a [128, n] fp32 tile, DVE-only (no ACT table).

    Quake-style bit trick seed + 3 Newton iterations.
    """
    i32 = mybir.dt.int32
    t_i = pool.tile([128, n], i32, tag="rs_i")
    r = pool.tile([128, n], F32, tag="rs_r")
    t1 = pool.tile([128, n], F32, tag="rs_t1")
    nc.vector.tensor_scalar(
        t_i[:], vpe.bitcast(i32), 1, None, mybir.AluOpType.arith_shift_right
    )
    nc.vector.tensor_scalar(
        r[:].bitcast(i32),
        t_i[:],
        -1,
        0x5F3759DF,
        mybir.AluOpType.mult,
        mybir.AluOpType.add,
    )
    for _ in range(3):
        nc.vector.tensor_tensor(t1[:], r[:], r[:], mybir.AluOpType.mult)
        nc.vector.tensor_tensor(t1[:], t1[:], vpe, mybir.AluOpType.mult)
        nc.vector.tensor_scalar(
            t1[:], t1[:], -0.5, 1.5, mybir.AluOpType.mult, mybir.AluOpType.add
        )
        nc.vector.tensor_tensor(r[:], r[:], t1[:], mybir.AluOpType.mult)
    return r


def emit_proj(nc, b, tt, yT, wp_sb, bp8_sb, acc_ps, out_pool, d_out):
    tsl = slice(tt * QT, (tt + 1) * QT)
    for ec in range(8):
        ps_p = acc_ps.tile([128, 512], F32, tag="acc", name="ps_p")
        nc.tensor.matmul(
            ps_p[:],
            wp_sb[:, ec * 128 : (ec + 1) * 128],
            yT[:, tsl],
            start=True,
            stop=True,
        )
        o_sb = out_pool.tile([128, 512], F32, tag="o")
        nc.scalar.activation(
            out=o_sb[:],
            in_=ps_p[:],
            func=mybir.ActivationFunctionType.Identity,
            bias=bp8_sb[:, ec : ec + 1],
            scale=1.0,
        )
        nc.sync.dma_start(
            d_out.ap()[
                ec * 128 : (ec + 1) * 128,
                b * T + tt * QT : b * T + (tt + 1) * QT,
            ],
            o_sb[:],
        )


def build_bass():
    nc = bacc.Bacc("TRN2", target_bir_lowering=False, debug=False, num_devices=N_CORES)

    d_xt = nc.dram_tensor("xt", [C_IN, BT], QDT, kind="ExternalInput")
    d_xbf = nc.dram_tensor("xbf", [BT, C_IN], BF16, kind="ExternalInput")
    d_w = nc.dram_tensor("wattn", [C_IN, COLS], QDT, kind="ExternalInput")
    d_sbc = nc.dram_tensor("sbc", [128, COLS], F32, kind="ExternalInput")
    d_bab = nc.dram_tensor("bab", [128, COLS], F32, kind="ExternalInput")
    d_wp = nc.dram_tensor("wp", [128, N_EMBD], MMDT, kind="ExternalInput")
    d_bp8 = nc.dram_tensor("bp8", [128, 8], F32, kind="ExternalInput")
    d_masks = nc.dram_tensor("masks", [4, 128, QT], MMDT, kind="ExternalInput")
    d_ident = nc.dram_tensor("ident", [128, 128], MMDT, kind="ExternalInput")
    d_ones = nc.dram_tensor("onesm", [128, 128], MMDT, kind="ExternalInput")
    d_out = nc.dram_tensor("out", [N_EMBD, BT], F32, kind="ExternalOutput")
    if K_DBG:
        d_dbgq = nc.dram_tensor("dbgq", [128, T], MMDT, kind="ExternalOutput")
        d_dbgk = nc.dram_tensor("dbgk", [128, T], MMDT, kind="ExternalOutput")
        d_dbgy = nc.dram_tensor("dbgy", [128, T], MMDT, kind="ExternalOutput")
        d_dbgv = nc.dram_tensor("dbgv", [128, TCH_PER_B, 72], MMDT, kind="ExternalOutput")

    with tile.TileContext(nc) as tc, ExitStack() as ctx:
        consts = ctx.enter_context(tc.tile_pool(name="consts", bufs=1))
        xt_pool = ctx.enter_context(tc.tile_pool(name="xt", bufs=4))
        xbf_pool = ctx.enter_context(tc.tile_pool(name="xbf", bufs=4))
        bn_pool = ctx.enter_context(tc.tile_pool(name="bn", bufs=4))
        st_pool = ctx.enter_context(tc.tile_pool(name="st", bufs=3))
        tmp_pool = ctx.enter_context(tc.tile_pool(name="tmp", bufs=4))
        qkv_pool = ctx.enter_context(tc.tile_pool(name="qkv", bufs=4))
        perb_pool = ctx.enter_context(tc.tile_pool(name="perb", bufs=3))
        exp_pool = ctx.enter_context(tc.tile_pool(name="expp", bufs=8))
        nrm_pool = ctx.enter_context(tc.tile_pool(name="nrm", bufs=4))
        out_pool = ctx.enter_context(tc.tile_pool(name="outp", bufs=4))
        acc_ps = ctx.enter_context(tc.tile_pool(name="accps", bufs=2, space="PSUM"))
        s_ps = ctx.enter_context(tc.tile_pool(name="sps", bufs=2, space="PSUM"))
        y_ps = ctx.enter_context(tc.tile_pool(name="yps", bufs=2, space="PSUM"))

        # --- constants ---
        w_sb = consts.tile([128, CC, COLS], QDT)
        nc.sync.dma_start(w_sb[:], d_w.ap().rearrange("(cc p) j -> p cc j", p=128))
        sbc_sb = consts.tile([128, COLS], F32)
        nc.sync.dma_start(sbc_sb[:], d_sbc.ap())
        bab_sb = consts.tile([128, COLS], F32)
        nc.sync.dma_start(bab_sb[:], d_bab.ap())
        wp_sb = consts.tile([128, N_EMBD], MMDT)
        nc.sync.dma_start(wp_sb[:], d_wp.ap())
        bp8_sb = consts.tile([128, 8], F32)
        nc.sync.dma_start(bp8_sb[:], d_bp8.ap())
        mask_sb = consts.tile([128, 4, QT], MMDT)
        nc.sync.dma_start(mask_sb[:], d_masks.ap().rearrange("m p q -> p m q"))
        ident_sb = consts.tile([128, 128], MMDT)
        nc.sync.dma_start(ident_sb[:], d_ident.ap())
        ones_sb = consts.tile([128, 128], MMDT)
        nc.sync.dma_start(ones_sb[:], d_ones.ap())

        xbf_v = d_xbf.ap().rearrange("(n p) c -> n p c", p=128)
        xt_v = d_xt.ap().rearrange("(cc p) t -> p cc t", p=128)

        def stream_b(b):
            """Generator emitting one batch's full pipeline; yields define
            interleave points for round-robin co-scheduling of two batches
            (fills PE dependency gaps with independent work)."""
            # ---------- Phase A: LN stats for this b ----------
            # rstd is computed in two half-batches so the first QKV
            # corrections only wait on 8 stats chunks, not 16
            stats = st_pool.tile([128, TCH_PER_B, 2], F32, tag="stats")
            nrstd = st_pool.tile([128, TCH_PER_B], F32, tag="nrstd")
            HALF = 4
            for i in range(TCH_PER_B):
                tci = b * TCH_PER_B + i
                xbf_t = xbf_pool.tile([128, C_IN], BF16)
                nc.sync.dma_start(xbf_t[:], xbf_v[tci])
                bn6 = bn_pool.tile([128, 3, 6], F32)
                xg = xbf_t[:].rearrange("p (g f) -> p g f", g=3)
                for g in range(3):
                    nc.vector.bn_stats(out=bn6[:, g, :], in_=xg[:, g, :])
                nc.vector.bn_aggr(out=stats[:, i, :], in_=bn6[:])
                if i % HALF == HALF - 1:
                    hsl = slice(i + 1 - HALF, i + 1)
                    vpe = st_pool.tile([128, HALF], F32, tag="vpe")
                    nc.vector.tensor_scalar(
                        vpe[:], stats[:, hsl, 1], EPS, None, mybir.AluOpType.add
                    )
                    rstd = _magic_rsqrt(nc, st_pool, vpe[:], HALF)
                    nc.vector.tensor_scalar(
                        nrstd[:, hsl], rstd[:], -1.0, None, mybir.AluOpType.mult
                    )
                if i % 4 == 3:
                    yield
            yield

            # ---------- Phase B: QKV + corrections + transposes ----------
            qT = perb_pool.tile([128, T], MMDT, tag="qT")
            kT = perb_pool.tile([128, T], MMDT, tag="kT")
            # 72-elem stride keeps every per-chunk V lhsT 16B-aligned
            vA = perb_pool.tile([128, TCH_PER_B, 72], MMDT, tag="vA")
            vB = perb_pool.tile([128, TCH_PER_B, 72], MMDT, tag="vB")
            def emit_qkv(i):
                tci = b * TCH_PER_B + i
                xt_t = xt_pool.tile([128, CC, 128], QDT)
                nc.gpsimd.dma_start(xt_t[:], xt_v[:, :, tci * 128 : (tci + 1) * 128])
                ps_qkv = acc_ps.tile([128, 512], F32, tag="acc")
                for cc in range(CC):
                    nc.tensor.matmul(
                        ps_qkv[:, :COLS],
                        xt_t[:, cc, :],
                        w_sb[:, cc, :],
                        start=(cc == 0),
                        stop=(cc == CC - 1),
                    )
                # corrections: qkv = (G - mu*s)*rstd + ba
                tmp = tmp_pool.tile([128, COLS], F32, tag="ctmp")
                nc.vector.scalar_tensor_tensor(
                    out=tmp[:],
                    in0=sbc_sb[:],
                    scalar=stats[:, i, 0:1],
                    in1=ps_qkv[:, :COLS],
                    op0=mybir.AluOpType.mult,
                    op1=mybir.AluOpType.subtract,
                )
                qkv_sb = qkv_pool.tile([128, COLS], MMDT, tag="qkv")
                nc.vector.scalar_tensor_tensor(
                    out=qkv_sb[:],
                    in0=tmp[:],
                    scalar=nrstd[:, i : i + 1],
                    in1=bab_sb[:],
                    op0=mybir.AluOpType.mult,
                    op1=mybir.AluOpType.add,
                )
                return qkv_sb

            def emit_tr(i, qkv_sb):
                # v slices (+ ones cols) for PV lhsT
                nc.vector.tensor_copy(out=vA[:, i, 0:64], in_=qkv_sb[:, 256:320])
                nc.vector.tensor_copy(out=vB[:, i, 0:64], in_=qkv_sb[:, 320:384])
                nc.vector.tensor_copy(out=vA[:, i, 64:65], in_=ones_sb[:, 0:1])
                nc.vector.tensor_copy(out=vB[:, i, 64:65], in_=ones_sb[:, 1:2])
                # transpose q and k 128x128 blocks -> [cols, tok]
                ps_tq = s_ps.tile([128, 128], MMDT, tag="sp", name="ps_tq")
                nc.tensor.transpose(ps_tq[:], qkv_sb[:, 0:128], ident_sb[:])
                nc.vector.tensor_copy(out=qT[:, i * 128 : (i + 1) * 128], in_=ps_tq[:])
                ps_tk = s_ps.tile([128, 128], MMDT, tag="sp", name="ps_tk")
                nc.tensor.transpose(ps_tk[:], qkv_sb[:, 128:256], ident_sb[:])
                nc.vector.tensor_copy(out=kT[:, i * 128 : (i + 1) * 128], in_=ps_tk[:])

            pend_b = []
            for i in range(TCH_PER_B):
                pend_b.append((i, emit_qkv(i)))
                if len(pend_b) > 1:
                    emit_tr(*pend_b.pop(0))
                yield
            for item in pend_b:
                emit_tr(*item)
            yield

            if K_DBG and b == 0:
                nc.sync.dma_start(d_dbgq.ap(), qT[:])
                nc.sync.dma_start(d_dbgk.ap(), kT[:])
                nc.sync.dma_start(d_dbgv.ap(), vA[:])

            # ---------- Phase C: attention ----------
            yT = perb_pool.tile([128, T], MMDT, tag="yT")
            for jt in range(NJT):
                nkc = 4 * (jt + 1)
                ps_yA = y_ps.tile([65, QT], F32, tag="y", name="ps_yA")
                ps_yB = y_ps.tile([65, QT], F32, tag="y", name="ps_yB")
                qsl = slice(jt * QT, (jt + 1) * QT)
                AHEAD = int(os.environ.get("K_AHEAD", "3"))  # scores/exp lead over PV

                def emit_scores(kc):
                    ksl = slice(kc * 128, (kc + 1) * 128)
                    off = kc * 128 - jt * QT
                    # both heads' scores go into one 2-bank psum tile so a
                    # single exp call covers them (amortizes ACT startup)
                    ps_s2 = s_ps.tile([128, 2 * QT], F32, tag="sp", name="ps_s2")
                    for h in range(2):
                        hp = slice(h * 64, (h + 1) * 64)
                        hsl = slice(h * QT, (h + 1) * QT)
                        if off >= 0:
                            # seed psum with the -1e9 causal mask (exp -> 0),
                            # then accumulate k^T q on top: masking stays on
                            # the PE stream instead of a DVE hop
                            nc.tensor.matmul(
                                ps_s2[:, hsl],
                                ident_sb[:],
                                mask_sb[:, off // 128, :],
                                start=True,
                                stop=False,
                            )
                        nc.tensor.matmul(
                            ps_s2[:, hsl],
                            kT[hp, ksl],
                            qT[hp, qsl],
                            start=(off < 0),
                            stop=True,
                        )
                    p_sb2 = exp_pool.tile([128, 2 * QT], MMDT, tag="p")
                    nc.scalar.activation(
                        out=p_sb2[:],
                        in_=ps_s2[:],
                        func=mybir.ActivationFunctionType.Exp,
                        scale=0.125,
                    )
                    return p_sb2

                def emit_pv(kc, p_sb2):
                    for h, (ps_y, v_t) in enumerate(((ps_yA, vA), (ps_yB, vB))):
                        nc.tensor.matmul(
                            ps_y[:],
                            v_t[:, kc, 0:65],
                            p_sb2[:, h * QT : (h + 1) * QT],
                            start=(kc == 0),
                            stop=(kc == nkc - 1),
                        )

                pending = []
                for kc in range(nkc):
                    pending.append((kc, emit_scores(kc)))
                    if len(pending) > AHEAD:
                        emit_pv(*pending.pop(0))
                    yield
                for item in pending:
                    emit_pv(*item)
                yield
                # Copy y_aug off PSUM right away (frees the accumulation bank
                # for the next q-tile), then normalize SBUF-side off the
                # critical path: y = y_aug[0:64] * (1/d), d = y_aug[64].
                ysbs = []
                for h, ps_y in enumerate((ps_yA, ps_yB)):
                    ysb = nrm_pool.tile([65, QT], F32, tag="ysb", bufs=4)
                    nc.vector.tensor_copy(out=ysb[:], in_=ps_y[:])
                    ysbs.append(ysb)
                # batch both heads' 1/d into one reciprocal call (it has a
                # large fixed cost); rows staged at partitions 0/1 via DMA
                dstage = nrm_pool.tile([2, QT], F32, tag="dstage")
                nc.sync.dma_start(dstage[0:1, :], ysbs[0][64:65, :])
                nc.sync.dma_start(dstage[1:2, :], ysbs[1][64:65, :])
                rsb2 = nrm_pool.tile([2, QT], F32, tag="rsb")
                nc.vector.reciprocal(rsb2[:], dstage[:])
                rsb_b1 = nrm_pool.tile([1, QT], F32, tag="rsb1")
                nc.sync.dma_start(rsb_b1[:], rsb2[1:2, :])
                for h, ysb in enumerate(ysbs):
                    rb_sb = nrm_pool.tile([64, QT], F32, tag="rb")
                    nc.gpsimd.partition_broadcast(
                        rb_sb[:], rsb2[0:1, :] if h == 0 else rsb_b1[0:1, :]
                    )
                    if h == 0:
                        nc.vector.tensor_tensor(
                            yT[0:64, qsl], ysb[0:64, :], rb_sb[:],
                            mybir.AluOpType.mult,
                        )
                    else:
                        yB_sb = nrm_pool.tile([64, QT], MMDT, tag="yB")
                        nc.vector.tensor_tensor(
                            yB_sb[:], ysb[0:64, :], rb_sb[:], mybir.AluOpType.mult
                        )
                        nc.sync.dma_start(yT[64:128, qsl], yB_sb[:])

                # projection pipelined one q-tile behind (deps long ready →
                # no head-of-line blocking on PE)
                if jt > 0:
                    emit_proj(nc, b, jt - 1, yT, wp_sb, bp8_sb, acc_ps, out_pool, d_out)
                yield
            emit_proj(nc, b, NJT - 1, yT, wp_sb, bp8_sb, acc_ps, out_pool, d_out)

            if K_DBG and b == 0:
                nc.sync.dma_start(d_dbgy.ap(), yT[:])

        # round-robin batch streams so independent matmuls fill each
        # other's dependency gaps in the static per-engine order
        n_active = int(os.environ.get("K_STREAMS", "1"))
        active = []
        next_b = 0
        while active or next_b < B:
            while len(active) < n_active and next_b < B:
                active.append(stream_b(next_b))
                next_b += 1
            for s in list(active):
                try:
                    next(s)
                except StopIteration:
                    active.remove(s)

    nc.compile()
    return nc


def _host_prep(x, ln_w, ln_b, W_attn, b_attn, W_proj, b_proj):
    x2d = np.asarray(x, np.float32).reshape(BT, C_IN)
    xt = np.ascontiguousarray(x2d.T).astype(MMNP)
    xbf = x2d.astype(ml_dtypes.bfloat16)
    Wf = np.asarray(ln_w, np.float32)[:, None] * np.asarray(W_attn, np.float32)
    ba_eff = np.asarray(b_attn, np.float32) + np.asarray(
        ln_b, np.float32
    ) @ np.asarray(W_attn, np.float32)

    # additive causal masks: 0 where k <= q, -1e9 (-> exp==0) where masked
    masks = np.zeros((4, 128, QT), np.float32)
    kk = np.arange(128)[:, None]
    qq = np.arange(QT)[None, :]
    for m in range(4):
        masks[m] = np.where(kk + m * 128 <= qq, 0.0, -1e9).astype(np.float32)
    ident = np.eye(128, dtype=np.float32)
    onesm = np.ones((128, 128), np.float32)

    in_maps = []
    for c in range(N_CORES):
        csl = slice(c * 128, (c + 1) * 128)
        qcols = np.r_[csl]
        cols = np.concatenate([qcols, qcols + N_EMBD, qcols + 2 * N_EMBD])
        Wc = np.ascontiguousarray(Wf[:, cols])
        s_c = Wc.sum(axis=0)
        ba_c = ba_eff[cols]
        in_maps.append(
            {
                "xt": xt,
                "xbf": xbf,
                "wattn": Wc.astype(MMNP),
                "sbc": np.ascontiguousarray(np.broadcast_to(s_c, (128, COLS))),
                "bab": np.ascontiguousarray(np.broadcast_to(ba_c, (128, COLS))),
                "wp": np.ascontiguousarray(
                    np.asarray(W_proj, np.float32)[csl, :]
                ).astype(MMNP),
                "bp8": np.ascontiguousarray(
                    np.asarray(b_proj, np.float32).reshape(8, 128).T / 8.0
                ),
                "masks": masks.astype(MMNP),
                "ident": ident.astype(MMNP),
                "onesm": onesm.astype(MMNP),
            }
        )
    return in_maps


def kernel(x, ln_w, ln_b, W_attn, b_attn, W_proj, b_proj):
    global _CACHED_NC, LAST_RESULTS
    if _CACHED_NC is None:
        _CACHED_NC = build_bass()
    in_maps = _host_prep(x, ln_w, ln_b, W_attn, b_attn, W_proj, b_proj)
    res = run_bass_kernel_spmd(_CACHED_NC, in_maps, core_ids=list(range(N_CORES)))
    LAST_RESULTS = res
    total = np.zeros((N_EMBD, BT), np.float64)
    for r in res.results:
        total += r["out"].astype(np.float64)
    out = total.T.astype(np.float32).reshape(B, T, N_EMBD)
    return out
